# revision 12
# baseline (speedup 1.0000x reference)
"""DistMult scoring kernel for Trainium2 (8 NeuronCores, Bass/Tile).

reference computation:
    rel = rel_embeds[rel_ids]                      # [B, D] gather
    scores = sum(head * rel * tail, axis=-1)       # [B]
    pos = min(scores[:n_pos], upper_bound)
    neg = max(scores[n_pos:], lower_bound)
    out = sigmoid(concat(pos, neg))

Strategy (matmul-scored, sorted batches):
  * Host sorts all B rows by rel_id and pads each rel group to a multiple
    of 128 rows, so every 128-row "batch" uses exactly ONE rel vector.
    Batches are dealt contiguously to 8 cores; outputs are unpermuted on
    the host (row order is free to choose since the final gather is ours).
  * Rows are laid out TRANSPOSED on device: d on partitions (2 chunks of
    128), rows on the free axis. h and t are stored int8 (x32) in DRAM and
    upcast to bf16 during the SWDGE DMA (halves HBM traffic; DVE needs
    16-bit for its 2x mode).
  * DVE does a single tensor_tensor pass q = h*t (bf16, 2x mode).
  * Per batch b, the score reduction over d is TWO matmuls on the (idle)
    tensor engine: lhsT = q_chunk[:, b*128:(b+1)*128] (stationary),
    rhs = g[:, 2b+c] (the batch's rel vector chunk, moving, N=1),
    accumulating into psum[:, b]. Measured issue rate ~33ns per pair.
  * psum[i, b] = 1024 * score(row 128b + i). Clamp against host-prescaled
    bounds (x1024, +/-inf on the inactive side), then sigmoid via the
    scalar engine with scale=2^-10. One output DMA; host unpermutes.
"""

import sys

for _p in ("/opt/trn_rl_repo",):
    if _p not in sys.path:
        sys.path.insert(0, _p)

import numpy as np

import concourse.bacc as bacc
import concourse.bass as bass
import concourse.mybir as mybir
import concourse.tile as tile
from concourse.bass_utils import run_bass_kernel_spmd

N_POS = 131072
N_NEG = 393216
B = N_POS + N_NEG  # 524288
D = 256
NUM_REL = 500
NCORES = 8
P = 128
W = 128          # rows per batch (one rel vector per batch)
GB = 48          # batches per group (48*128 = 6144 rows per group-chunk tile)
CAST_B = 18      # batches per group on the cast-DMA + DVE-bf16 (2x) path
DVE8_B = 23      # batches per group on the raw-int8 DVE (1x) path
POOL_B = 7       # batches per group on the raw-int8 GPSIMD path
HSCALE = 32.0    # h, t int8 quantization scale; scores come out x1024
SSCALE = float(HSCALE * HSCALE)


def build_program(nb: int):
    """nb: batches per core (each 128 rows, single rel)."""
    f32 = mybir.dt.float32
    i8 = mybir.dt.int8
    bf = mybir.dt.bfloat16
    mult = mybir.AluOpType.mult

    rows = nb * W
    ngroups = (nb + GB - 1) // GB

    nc = bacc.Bacc(
        "TRN2", target_bir_lowering=False, debug=False, num_devices=NCORES
    )
    # transposed int8 streams: [chunk, d, row]
    h8 = nc.declare_dram_parameter("h8", [2, P, rows], i8, isOutput=False)
    t8 = nc.declare_dram_parameter("t8", [2, P, rows], i8, isOutput=False)
    # per-batch rel vectors: g[d, 2b+c] = rel_vec(b)[128c + d]
    g = nc.declare_dram_parameter("g", [P, 2 * nb], bf, isOutput=False)
    ub = nc.declare_dram_parameter("ub", [P, nb], f32, isOutput=False)
    lb = nc.declare_dram_parameter("lb", [P, nb], f32, isOutput=False)
    out = nc.declare_dram_parameter("out", [P, nb], f32, isOutput=True)

    with tile.TileContext(nc) as tc:
        with (
            tc.tile_pool(name="io", bufs=1) as io_pool,
            tc.tile_pool(name="stream", bufs=3) as spool,
            tc.tile_pool(name="psum", bufs=4, space="PSUM") as psum_pool,
            tc.tile_pool(name="scratch", bufs=2) as qpool,
        ):
            gt = io_pool.tile([P, 2 * nb], bf)
            nc.sync.dma_start(out=gt[:], in_=g[:])
            ubt = io_pool.tile([P, nb], f32)
            nc.sync.dma_start(out=ubt[:], in_=ub[:])
            lbt = io_pool.tile([P, nb], f32)
            nc.sync.dma_start(out=lbt[:], in_=lb[:])
            scores = io_pool.tile([P, nb], f32)

            # group size schedule: small first groups shorten the pipeline head
            sizes = []
            remaining = nb
            for sz in (16, 32):
                if remaining > sz:
                    sizes.append(sz)
                    remaining -= sz
            while remaining > 0:
                sz = min(GB, remaining)
                sizes.append(sz)
                remaining -= sz

            CW = CAST_B * W
            IW = (DVE8_B + POOL_B) * W
            b0 = 0
            for gi, gb in enumerate(sizes):
                r0 = b0 * W
                gw = gb * W
                # per-group 3-way split, scaled to group size
                cb = (gb * CAST_B) // GB
                db = (gb * DVE8_B) // GB
                pb = gb - cb - db
                cw = cb * W
                dw = db * W
                iw = (db + pb) * W
                hb = [None, None]
                tb = [None, None]
                h8t = [None, None]
                t8t = [None, None]
                qp = [None, None]
                for c in range(2):
                    hb[c] = spool.tile([P, CW], bf, tag=f"h{c}", name=f"hb{c}")
                    tb[c] = spool.tile([P, CW], bf, tag=f"t{c}", name=f"tb{c}")
                    if cb > 0:
                        nc.gpsimd.dma_start(
                            out=hb[c][:, :cw], in_=h8[c, :, r0 : r0 + cw]
                        )
                        nc.gpsimd.dma_start(
                            out=tb[c][:, :cw], in_=t8[c, :, r0 : r0 + cw]
                        )
                    h8t[c] = spool.tile([P, IW], i8, tag=f"h8{c}", name=f"h8t{c}")
                    t8t[c] = spool.tile([P, IW], i8, tag=f"t8{c}", name=f"t8t{c}")
                    qp[c] = spool.tile([P, IW], bf, tag=f"qp{c}", name=f"qp{c}")
                    if iw > 0:
                        nc.sync.dma_start(
                            out=h8t[c][:, :iw], in_=h8[c, :, r0 + cw : r0 + gw]
                        )
                        nc.sync.dma_start(
                            out=t8t[c][:, :iw], in_=t8[c, :, r0 + cw : r0 + gw]
                        )
                for c in range(2):
                    if cb > 0:
                        nc.vector.tensor_tensor(
                            out=hb[c][:, :cw], in0=hb[c][:, :cw],
                            in1=tb[c][:, :cw], op=mult,
                        )
                    if db > 0:
                        nc.vector.tensor_tensor(
                            out=qp[c][:, :dw], in0=h8t[c][:, :dw],
                            in1=t8t[c][:, :dw], op=mult,
                        )
                    if pb > 0:
                        nc.gpsimd.tensor_tensor(
                            out=qp[c][:, dw:iw], in0=h8t[c][:, dw:iw],
                            in1=t8t[c][:, dw:iw], op=mult,
                        )
                # per-batch score reduction on the tensor engine
                ps = psum_pool.tile([P, GB], f32, tag="ps")
                for b in range(gb):
                    for c in range(2):
                        if b < cb:
                            lhsT = hb[c][:, b * W : (b + 1) * W]
                        else:
                            lhsT = qp[c][:, (b - cb) * W : (b - cb + 1) * W]
                        nc.tensor.matmul(
                            out=ps[:, b : b + 1],
                            lhsT=lhsT,
                            rhs=gt[:, 2 * (b0 + b) + c : 2 * (b0 + b) + c + 1],
                            start=(c == 0),
                            stop=(c == 1),
                        )
                # clamp into the persistent scores tile
                clip = qpool.tile([P, GB], f32, tag="clip")
                nc.vector.tensor_tensor(
                    out=clip[:, :gb], in0=ps[:, :gb], in1=ubt[:, b0 : b0 + gb],
                    op=mybir.AluOpType.min,
                )
                nc.vector.tensor_tensor(
                    out=scores[:, b0 : b0 + gb], in0=clip[:, :gb],
                    in1=lbt[:, b0 : b0 + gb], op=mybir.AluOpType.max,
                )
                b0 += gb

            sig = io_pool.tile([P, nb], f32)
            nc.scalar.activation(
                out=sig[:], in_=scores[:],
                func=mybir.ActivationFunctionType.Sigmoid,
                scale=1.0 / SSCALE,
            )
            nc.sync.dma_start(out=out[:], in_=sig[:])

    nc.compile()
    return nc


def make_in_maps(inputs: dict):
    import ml_dtypes

    bf16 = ml_dtypes.bfloat16

    head = np.asarray(inputs["head_embeds"], dtype=np.float32)
    tail = np.asarray(inputs["tail_embeds"], dtype=np.float32)
    rel_ids = np.asarray(inputs["rel_ids"]).astype(np.int64)
    lower = np.asarray(inputs["lower_bound"], dtype=np.float32)
    upper = np.asarray(inputs["upper_bound"], dtype=np.float32)
    table = np.asarray(inputs["rel_embeds"], dtype=np.float32)

    # --- sort rows by rel id, pad each rel group to a multiple of W rows
    order = np.argsort(rel_ids, kind="stable")
    sorted_ids = rel_ids[order]
    counts = np.bincount(sorted_ids, minlength=NUM_REL)
    padded = ((counts + W - 1) // W) * W
    total_batches = int(padded.sum()) // W
    nb = -(-total_batches // NCORES)  # ceil
    nbatch_total = nb * NCORES

    # row_src[j] = original row index or -1 (pad); batch_rel[b] = rel id
    row_src = np.full(nbatch_total * W, -1, dtype=np.int64)
    batch_rel = np.zeros(nbatch_total, dtype=np.int64)
    src_ofs = 0
    dst_ofs = 0
    bidx = 0
    for k in range(NUM_REL):
        n = int(counts[k])
        pn = int(padded[k])
        if pn == 0:
            continue
        row_src[dst_ofs : dst_ofs + n] = order[src_ofs : src_ofs + n]
        batch_rel[bidx : bidx + pn // W] = k
        src_ofs += n
        dst_ofs += pn
        bidx += pn // W

    # --- quantize h, t to int8 (x32) and build per-core transposed streams
    h8 = np.clip(np.round(head * HSCALE), -127, 127).astype(np.int8)
    t8 = np.clip(np.round(tail * HSCALE), -127, 127).astype(np.int8)
    src = row_src.copy()
    pad_mask = src < 0
    src[pad_mask] = 0
    h8_s = h8[src]
    t8_s = t8[src]
    h8_s[pad_mask] = 0
    t8_s[pad_mask] = 0

    # bounds in score space (x1024), +/-inf on the inactive side
    ubf = np.full(nbatch_total * W, np.inf, dtype=np.float32)
    lbf = np.full(nbatch_total * W, -np.inf, dtype=np.float32)
    pos_rows = (row_src >= 0) & (row_src < N_POS)
    neg_rows = row_src >= N_POS
    ubf[pos_rows] = upper[row_src[pos_rows]] * SSCALE
    lbf[neg_rows] = lower[row_src[neg_rows] - N_POS] * SSCALE

    table_bf = table.astype(bf16)

    rows = nb * W
    in_maps = []
    for c in range(NCORES):
        r0 = c * rows
        r1 = r0 + rows
        # [rows, 256] -> [256, rows] -> [2, 128, rows]
        hc = np.ascontiguousarray(h8_s[r0:r1].T).reshape(2, P, rows)
        tc_ = np.ascontiguousarray(t8_s[r0:r1].T).reshape(2, P, rows)
        # g[d, 2b+c] = table[rel(b), 128c+d]
        rel_c = batch_rel[c * nb : (c + 1) * nb]
        gc = table_bf[rel_c].reshape(nb, 2, P)  # [b, chunk, d]
        gc = np.ascontiguousarray(gc.transpose(2, 0, 1).reshape(P, 2 * nb))
        # bounds laid [i, b]: row j = 128b + i
        ub_c = np.ascontiguousarray(ubf[r0:r1].reshape(nb, W).T)
        lb_c = np.ascontiguousarray(lbf[r0:r1].reshape(nb, W).T)
        in_maps.append(
            {"h8": hc, "t8": tc_, "g": gc, "ub": ub_c, "lb": lb_c}
        )
    return in_maps, nb, row_src


def assemble_output(results, nb: int, row_src: np.ndarray) -> np.ndarray:
    rows = nb * W
    full = np.empty(B, dtype=np.float32)
    for c in range(NCORES):
        res = np.asarray(results[c]["out"], dtype=np.float32)  # [128, nb]
        flat = res.T.reshape(-1)  # j order: j = 128*b + i
        src = row_src[c * rows : (c + 1) * rows]
        m = src >= 0
        full[src[m]] = flat[m]
    return full


def kernel(**inputs) -> np.ndarray:
    in_maps, nb, row_src = make_in_maps(inputs)
    nc = build_program(nb)
    res = run_bass_kernel_spmd(nc, in_maps, list(range(NCORES)))
    return assemble_output(res.results, nb, row_src)
